# revision 3
# baseline (speedup 1.0000x reference)
"""Trainium2 Bass kernel for nn_Network_81862076662591 (sampling network).

Self-contained: takes FULL inputs (as produced by the problem's
setup_inputs), data-parallel shards batch B=256 over 8 NeuronCores
(32 rows each, per-iteration weights replicated), runs the fused
LSTM + gumbel-argmax sampling + MLP scan on-device, and returns the
full [256, 10, 100] output.

Per-core design (B=32, T=100, D=784=7x112, G=256, H=128, fp32):
  - per-iteration weights prepacked host-side into two contiguous
    SBUF-layout images (megaA: W1+W2 on 112 partitions, megaB:
    W3/W4/Wf2/Wg1/Wg2 on 128 partitions) -> 3 large dma_starts per
    iteration instead of 8 strided ones;
  - activations feature-major [feat, batch] so weight-stationary
    matmuls chain without transposes; sampling runs batch-major via
    DVE max/max_index; the one-hot (scaled by -1e9) doubles as the
    negmem increment and is PE-transposed back feature-major;
  - st = hard + soft - stop_grad(soft) == hard numerically, so the
    softmax is skipped; zero biases are dropped (bg2 folded into the
    gumbel tensor host-side);
  - off-critical-path ops (negmem update, gum+negmem precompute) run
    on gpsimd;
  - everything stays fp32: the gumbel-argmax top-2 gap can be <1e-3,
    so fp16/bf16 weights risk argmax flips worth ~3e-2 rel err each.
"""
from contextlib import ExitStack

import numpy as np

import concourse.bass as bass
import concourse.mybir as mybir
import concourse.tile as tile
from concourse.vector_clock import ScopedClock
from concourse.bass_utils import run_bass_kernel_spmd

F32 = mybir.dt.float32
ALU = mybir.AluOpType
ACTF = mybir.ActivationFunctionType


class _TileContextSplitDrain(tile.TileContext):
    """This walrus build rejects >1 sem-wait on the kernel-tail Drain;
    split the accumulated waits across several sequential drains."""

    def _drain_and_barrier(self, tick_clock, wait_clock):
        drain_inst = self.nc.sync.drain()
        wait_clock.add_sem_waits(
            drain_inst.ins, ScopedClock({None: tick_clock.global_clock}))
        si = drain_inst.ins.sync_info
        waits = list(si.on_wait or []) if si is not None else []
        if len(waits) > 1:
            si.on_wait = [waits[0]]
            for w in waits[1:]:
                d2 = self.nc.sync.drain()
                if d2.ins.sync_info is None:
                    d2.ins.sync_info = mybir.SyncInfo(on_wait=[w], on_update=[])
                else:
                    d2.ins.sync_info.on_wait = [w]
        self.nc.all_engine_barrier()
        assert self.sems is not None
        popped = self.nc._tile_sem_poison_stack.pop()
        assert popped is self._sem_poison
        self.nc.clear_and_free_semaphores(list(self.sems.allocated().values()))
        self.nc.all_engine_barrier()


def _split_multi_waits(nc, limit=1):
    """This walrus accepts only `limit` sem-waits per instruction; move the
    excess onto same-engine sequencer NOPs inserted immediately before."""
    import copy

    proto = nc.vector.isa(nc.isa.Opcode.NEURON_ISA_TPB_OPCODE_NOP, {}).ins
    nop_ctr = [0]

    def make_nop(engine, waits):
        nop = copy.deepcopy(proto)
        nop_ctr[0] += 1
        nop.name = f"waitnop-{nop_ctr[0]}"
        nop.engine = engine
        nop.sync_info = mybir.SyncInfo(on_wait=list(waits), on_update=[])
        return nop

    skip = ("InstAllEngineBarrier", "InstEventSemaphore")
    for fn in nc.m.functions:
        for bb in fn.blocks:
            insts = bb.instructions
            if insts and insts[-1] is proto:
                insts.pop()
            out = []
            for inst in insts:
                si = inst.sync_info
                waits = list(si.on_wait or []) if si is not None else []
                if len(waits) > limit and type(inst).__name__ not in skip:
                    for i in range(0, len(waits) - limit, limit):
                        out.append(make_nop(inst.engine, waits[i:i + limit]))
                    si.on_wait = waits[len(waits) - limit:]
                out.append(inst)
            bb.instructions[:] = out



NCORES = 8
B = 32
D = 784
KP = 112         # W1/W2 contraction chunk rows (7*112 = 784)
G = 256
H = 128
T = 100
NEGBIG = -1.0e9

# megaA element offsets (fp32), partition dim 112
OFF_W1 = 0                     # [7k][784n]
OFF_W2 = OFF_W1 + 7 * D        # 5488  [7k][256n]
SZA = OFF_W2 + 7 * G           # 7280 elems -> 29120 B/partition

# megaB element offsets (fp32), partition dim 128
OFF_W3 = 0                     # [2k][128n]
OFF_W4 = OFF_W3 + 2 * H        # 256
OFF_WF2 = OFF_W4 + H           # 384  [16n]
OFF_WG1 = OFF_WF2 + 16         # 400  [256n]
OFF_WG2 = OFF_WG1 + G          # 656  [2k][784n]
SZB = OFF_WG2 + 2 * D          # 2224 elems -> 8896 B/partition


def _build(ctx, tc, p, w_bufs=3):
    nc = tc.nc

    cpool = ctx.enter_context(tc.tile_pool(name="const", bufs=1))
    spool = ctx.enter_context(tc.tile_pool(name="state", bufs=1))
    wpool = ctx.enter_context(tc.tile_pool(name="w", bufs=w_bufs))
    gpool = ctx.enter_context(tc.tile_pool(name="g", bufs=w_bufs))
    psum = ctx.enter_context(tc.tile_pool(name="ps", bufs=1, space="PSUM"))

    WihS = cpool.tile([H, 4 * H], F32, tag="wih")
    WhhS = cpool.tile([H, 4 * H], F32, tag="whh")
    IDENT = cpool.tile([B, B], F32, tag="ident")
    IOTA = cpool.tile([B, D], F32, tag="iota")
    XB = cpool.tile([B, D], F32, tag="xb")
    nc.sync.dma_start(WihS[:], p["wih"].ap())
    nc.sync.dma_start(WhhS[:], p["whh"].ap())
    nc.sync.dma_start(IDENT[:], p["ident"].ap())
    nc.sync.dma_start(IOTA[:], p["iota"].ap())
    nc.sync.dma_start(XB[:], p["x"].ap())

    A_fm = spool.tile([H, B], F32, tag="a")        # lin^T
    H_fm = spool.tile([H, B], F32, tag="h")
    C_fm = spool.tile([H, B], F32, tag="c")
    MEMFM = spool.tile([KP, 7, B], F32, tag="memfm")  # mask^T
    NEGMEM = spool.tile([B, D], F32, tag="negmem")
    XFM = spool.tile([KP, 7, B], F32, tag="xfm")      # x^T
    YT = spool.tile([KP, 7, B], F32, tag="yt")
    A1 = spool.tile([KP, 7, B], F32, tag="a1")
    A2 = spool.tile([H, 2, B], F32, tag="a2")
    A3 = spool.tile([H, B], F32, tag="a3")
    G1S = spool.tile([H, 2, B], F32, tag="g1")
    SC1 = spool.tile([KP, 7, B], F32, tag="sc1")
    SC2 = spool.tile([H, 2, B], F32, tag="sc2")
    SC3 = spool.tile([H, B], F32, tag="sc3")
    SI = spool.tile([H, 2, B], F32, tag="si")        # sig(i), sig(f)
    TG = spool.tile([H, B], F32, tag="tg")
    SO = spool.tile([H, B], F32, tag="so")
    TC = spool.tile([H, B], F32, tag="tc")
    U0 = spool.tile([H, B], F32, tag="u0")
    U1 = spool.tile([H, B], F32, tag="u1")
    HARD = spool.tile([B, D], F32, tag="hard")   # holds -1e9 * one_hot
    PERT = spool.tile([B, D], F32, tag="pert")
    MAX8 = spool.tile([B, 8], F32, tag="max8")
    IDX8 = spool.tile([B, 8], mybir.dt.uint32, tag="idx8")
    IDXF = spool.tile([B, 1], F32, tag="idxf")
    SOUT = spool.tile([B, 10, T], F32, tag="sout")

    nc.vector.memset(A_fm[:], 0.0)
    nc.vector.memset(H_fm[:], 0.0)
    nc.vector.memset(C_fm[:], 0.0)
    nc.vector.memset(MEMFM[:], 0.0)
    nc.vector.memset(NEGMEM[:], 0.0)

    ps_tr = psum.tile([KP, 7, B], F32, tag="tr")
    for k in range(7):
        nc.tensor.transpose(ps_tr[:, k, :], XB[:, k * KP:(k + 1) * KP],
                            IDENT[:])
    nc.scalar.copy(XFM[:], ps_tr[:])

    for t in range(T):
        MGA = wpool.tile([KP, SZA], F32, tag="megaA")
        MGB = wpool.tile([H, SZB], F32, tag="megaB")
        GUM = gpool.tile([B, D], F32, tag="gum")
        nc.sync.dma_start(MGA[:], p["megaA"].ap()[t])
        nc.sync.dma_start(MGB[:], p["megaB"].ap()[t])
        nc.sync.dma_start(GUM[:], p["gum"].ap()[t])
        # gum + negmem precompute on gpsimd (feeds PERT add below)
        GN = gpool.tile([B, D], F32, tag="gn")
        nc.gpsimd.tensor_tensor(GN[:], GUM[:], NEGMEM[:], ALU.add)

        # ---- LSTM cell (order i,f,g,o) ----
        ps_g = psum.tile([H, 4, B], F32, tag="g")
        for j in range(4):
            nc.tensor.matmul(ps_g[:, j, :], WihS[:, j * H:(j + 1) * H],
                             A_fm[:], start=True, stop=False)
            nc.tensor.matmul(ps_g[:, j, :], WhhS[:, j * H:(j + 1) * H],
                             H_fm[:], start=False, stop=True)
        nc.scalar.activation(SI[:], ps_g[:, 0:2, :], ACTF.Sigmoid)
        nc.scalar.activation(TG[:], ps_g[:, 2, :], ACTF.Tanh)
        nc.scalar.activation(SO[:], ps_g[:, 3, :], ACTF.Sigmoid)
        nc.vector.tensor_tensor(U0[:], SI[:, 1, :], C_fm[:], ALU.mult)
        nc.vector.tensor_tensor(U1[:], SI[:, 0, :], TG[:], ALU.mult)
        nc.vector.tensor_tensor(C_fm[:], U0[:], U1[:], ALU.add)
        nc.scalar.activation(TC[:], C_fm[:], ACTF.Tanh)
        nc.vector.tensor_tensor(H_fm[:], SO[:], TC[:], ALU.mult)

        # ---- gating MLP -> logits (batch-major) ----
        ps_g1 = psum.tile([H, 2, B], F32, tag="g1")
        for m in range(2):
            nc.tensor.matmul(ps_g1[:, m, :],
                             MGB[:, OFF_WG1 + m * H:OFF_WG1 + (m + 1) * H],
                             H_fm[:], start=True, stop=True)
        nc.scalar.copy(SC2[:], ps_g1[:])
        nc.vector.scalar_tensor_tensor(G1S[:], SC2[:], 0.2, SC2[:],
                                       ALU.mult, ALU.max)

        ps_lg = psum.tile([B, D], F32, tag="lg")
        for k in range(2):
            nc.tensor.matmul(ps_lg[:, 0:512], G1S[:, k, :],
                             MGB[:, OFF_WG2 + k * D:OFF_WG2 + k * D + 512],
                             start=(k == 0), stop=(k == 1))
        for k in range(2):
            nc.tensor.matmul(ps_lg[:, 512:D], G1S[:, k, :],
                             MGB[:, OFF_WG2 + k * D + 512:OFF_WG2 + (k + 1) * D],
                             start=(k == 0), stop=(k == 1))

        # ---- sampling ----
        nc.vector.tensor_tensor(PERT[:], ps_lg[:], GN[:], ALU.add)
        nc.vector.max(MAX8[:], PERT[:])
        nc.vector.max_index(IDX8[:], MAX8[:], PERT[:])
        nc.vector.tensor_copy(IDXF[:], IDX8[:, 0:1])
        nc.vector.tensor_scalar(HARD[:], IOTA[:], IDXF[:], NEGBIG,
                                ALU.is_equal, ALU.mult)
        nc.gpsimd.tensor_tensor(NEGMEM[:], HARD[:], NEGMEM[:], ALU.add)
        ps_tr = psum.tile([KP, 7, B], F32, tag="tr")
        for k in range(7):
            nc.tensor.transpose(ps_tr[:, k, :], HARD[:, k * KP:(k + 1) * KP],
                                IDENT[:])
        # mask^T += transpose(HARD) * (1/-1e9)
        nc.vector.scalar_tensor_tensor(MEMFM[:], ps_tr[:], 1.0 / NEGBIG,
                                       MEMFM[:], ALU.mult, ALU.add)
        nc.vector.tensor_tensor(YT[:], MEMFM[:], XFM[:], ALU.mult)

        # ---- f1 MLP ----
        ps_y1 = psum.tile([KP, 7, B], F32, tag="y1")
        for m in range(7):
            for k in range(7):
                nc.tensor.matmul(
                    ps_y1[:, m, :],
                    MGA[:, OFF_W1 + k * D + m * KP:OFF_W1 + k * D + (m + 1) * KP],
                    YT[:, k, :], start=(k == 0), stop=(k == 6))
        nc.scalar.copy(SC1[:], ps_y1[:])
        nc.vector.scalar_tensor_tensor(A1[:], SC1[:], 0.2, SC1[:],
                                       ALU.mult, ALU.max)

        ps_a2 = psum.tile([H, 2, B], F32, tag="a2")
        for m in range(2):
            for k in range(7):
                nc.tensor.matmul(
                    ps_a2[:, m, :],
                    MGA[:, OFF_W2 + k * G + m * H:OFF_W2 + k * G + (m + 1) * H],
                    A1[:, k, :], start=(k == 0), stop=(k == 6))
        nc.scalar.copy(SC2[:], ps_a2[:])
        nc.vector.scalar_tensor_tensor(A2[:], SC2[:], 0.2, SC2[:],
                                       ALU.mult, ALU.max)

        ps_a3 = psum.tile([H, B], F32, tag="tr")
        for k in range(2):
            nc.tensor.matmul(ps_a3[:],
                             MGB[:, OFF_W3 + k * H:OFF_W3 + (k + 1) * H],
                             A2[:, k, :], start=(k == 0), stop=(k == 1))
        nc.scalar.copy(SC3[:], ps_a3[:])
        nc.vector.scalar_tensor_tensor(A3[:], SC3[:], 0.2, SC3[:],
                                       ALU.mult, ALU.max)

        ps_a4 = psum.tile([H, B], F32, tag="g1")
        nc.tensor.matmul(ps_a4[:], MGB[:, OFF_W4:OFF_W4 + H], A3[:],
                         start=True, stop=True)
        nc.scalar.copy(SC3[:], ps_a4[:])
        nc.vector.scalar_tensor_tensor(A_fm[:], SC3[:], 0.2, SC3[:],
                                       ALU.mult, ALU.max)

        ps_s = psum.tile([B, 10], F32, tag="a2")
        nc.tensor.matmul(ps_s[:], A_fm[:], MGB[:, OFF_WF2:OFF_WF2 + 10],
                         start=True, stop=True)
        nc.scalar.copy(SOUT[:, :, t], ps_s[:])

    nc.sync.dma_start(p["out"].ap(), SOUT[:].rearrange("b c t -> b (c t)"))


def build_nc(w_bufs=3):
    nc = bass.Bass("TRN2", target_bir_lowering=False, debug=False)
    dp = nc.declare_dram_parameter
    p = {}
    p["megaA"] = dp("megaA", [T, KP, SZA], F32, isOutput=False)
    p["megaB"] = dp("megaB", [T, H, SZB], F32, isOutput=False)
    p["gum"] = dp("gum", [T, B, D], F32, isOutput=False)
    p["x"] = dp("x", [B, D], F32, isOutput=False)
    p["wih"] = dp("wih", [H, 4 * H], F32, isOutput=False)
    p["whh"] = dp("whh", [H, 4 * H], F32, isOutput=False)
    p["ident"] = dp("ident", [B, B], F32, isOutput=False)
    p["iota"] = dp("iota", [B, D], F32, isOutput=False)
    p["out"] = dp("out", [B, 10 * T], F32, isOutput=True)
    with _TileContextSplitDrain(nc) as tc:
        with ExitStack() as ctx:
            _build(ctx, tc, p, w_bufs=w_bufs)
    _split_multi_waits(nc)
    return nc


def prepack(inputs):
    f = lambda k: np.ascontiguousarray(np.asarray(inputs[k]), dtype=np.float32)
    for bn in ("b1", "b2", "b3", "b4", "bf2", "bg1", "bih", "bhh"):
        if bn in inputs and np.any(np.asarray(inputs[bn])):
            raise NotImplementedError(f"nonzero bias {bn} not supported")

    megaA = np.empty((T, KP, SZA), np.float32)
    megaB = np.zeros((T, H, SZB), np.float32)

    def put(dst, dst_off, w, pdim, kchunks, ncols):
        Tk, Kk, Nk = w.shape
        wp = np.zeros((T, kchunks * pdim, ncols), np.float32)
        wp[:, :Kk, :Nk] = w
        wp = wp.reshape(T, kchunks, pdim, ncols).transpose(0, 2, 1, 3)
        dst[:, :, dst_off:dst_off + kchunks * ncols] = \
            wp.reshape(T, pdim, kchunks * ncols)

    put(megaA, OFF_W1, f("W1"), KP, 7, D)
    put(megaA, OFF_W2, f("W2"), KP, 7, G)
    put(megaB, OFF_W3, f("W3"), H, 2, H)
    put(megaB, OFF_W4, f("W4"), H, 1, H)
    put(megaB, OFF_WF2, f("Wf2"), H, 1, 16)
    put(megaB, OFF_WG1, f("Wg1"), H, 1, G)
    put(megaB, OFF_WG2, f("Wg2"), H, 2, D)

    x = f("x")
    gum_all = f("gumbel") + f("bg2")[:, None, :]

    shared = {
        "megaA": megaA,
        "megaB": megaB,
        "wih": f("Wih"),
        "whh": f("Whh"),
        "ident": np.eye(B, dtype=np.float32),
        "iota": np.tile(np.arange(D, dtype=np.float32), (B, 1)),
    }
    in_maps = []
    for c in range(NCORES):
        sl = slice(c * B, (c + 1) * B)
        m = dict(shared)
        m["x"] = np.ascontiguousarray(x[sl])
        m["gum"] = np.ascontiguousarray(gum_all[:, sl])
        in_maps.append(m)
    return in_maps


def _add_nonce(nc):
    """Declare a dummy input whose SHAPE encodes the BIR digest: the PJRT
    module fingerprint ignores backend_config (where the BIR travels), so
    without this, kernels with identical I/O signatures but different
    bodies can collide in the neuron compile cache."""
    import hashlib
    h = hashlib.sha256()
    for fn in nc.m.functions:
        for bb in fn.blocks:
            for ins in bb.instructions:
                h.update(repr(ins).encode())
    d = int(h.hexdigest()[:16], 16)
    shape = [1 + d % 251, 1 + (d >> 16) % 241]
    nc.declare_dram_parameter("nonce", shape, F32, isOutput=False)
    return np.zeros(shape, np.float32)


_CACHE = {}


def _get_nc():
    if "nc" not in _CACHE:
        nc = build_nc()
        nonce = _add_nonce(nc)
        _CACHE["nc"] = (nc, nonce)
    return _CACHE["nc"]


def kernel(**inputs) -> np.ndarray:
    in_maps = prepack(inputs)
    nc, nonce = _get_nc()
    for m in in_maps:
        m["nonce"] = nonce
    res = run_bass_kernel_spmd(nc, in_maps, list(range(NCORES)))
    out = np.concatenate(
        [res.results[c]["out"].reshape(B, 10, T) for c in range(NCORES)],
        axis=0)
    return out.astype(np.float32)
